# revision 10
# baseline (speedup 1.0000x reference)
"""Trainium2 Bass kernel for nn_FEPSurrogateNetwork (GNN message passing).

Edge-parallel SPMD over 8 NeuronCores; v2 design:
  * Edges sorted by row, packed into 128-edge chunks on a static
    node-linear schedule (G nodes/chunk, WIN-node scatter window,
    SWIN-node expand window); 4 chunks = one 512-edge tile.
  * One-hot expand/scatter matrices generated ON-CHIP per tile (DVE
    is_equal against a resident iota + PE transpose) from a small
    resident rel-index table; scatter one-hot is a column slice of the
    expand one-hot.  dist/colidx tables are SBUF-resident (loaded once).
  * h[col] gathers batched: one indirect DMA per 4-tile group using a
    multi-column offset AP.
  * Layer-0 xloc (= h0 @ W1a0) and the padded h0_full gather table are
    host-precomputed inputs, so there is no initial AllGather.
  * Node MLP/LayerNorm blocks + next layer's xloc matmuls are emitted
    interleaved at superwindow-flush points, sharing PSUM tags with the
    edge pipeline; AllGather of the updated h slice per layer.
"""

import os
from contextlib import ExitStack

import numpy as np
import ml_dtypes

import concourse.bacc as bacc
import concourse.bass as bass
import concourse.mybir as mybir
import concourse.tile as tile
from concourse.bass_utils import run_bass_kernel_spmd
from concourse.masks import make_identity

BF16 = mybir.dt.bfloat16
F32 = mybir.dt.float32
I32 = mybir.dt.int32
AF = mybir.ActivationFunctionType
ALU = mybir.AluOpType
ts = bass.ts

P = 128


class Cfg:
    def __init__(self, N, E, HD, L, NC=8, G=10, CAP=128, WIN=64, SWIN=96, GT=4):
        assert HD == P
        self.N, self.E, self.HD, self.L, self.NC = N, E, HD, L, NC
        self.G, self.CAP, self.WIN, self.SWIN, self.GT = G, CAP, WIN, SWIN, GT
        self.NLOC = N // NC
        assert self.NLOC * NC == N
        self.NB = (self.NLOC + P - 1) // P  # node blocks per core
        self.NLOCP = self.NB * P
        ch = (self.NLOC + G - 1) // G + 2  # +2 slack chunks
        self.CHUNKS = (ch + 3) // 4 * 4
        self.TILES = self.CHUNKS // 4
        # scatter window start for chunk c (node coords, may be <0)
        self.o = lambda c: (c + 1) * G - WIN
        # expand window for tile t: SWIN rows from sbase(t)
        self.sbase = lambda t: 4 * G * t + G - WIN - 2
        assert 3 * G + WIN + 2 <= SWIN, "expand window too small"
        self.NSW = (self.o(self.CHUNKS - 1) + WIN + 511) // 512
        # xloc dram padding: lead so sbase+XPAD >= 0, tail so window end fits
        self.XPAD = WIN + 2
        self.XROWS = self.XPAD + self.NLOCP + SWIN + 64
        # packed dist table: tile t -> partition t % P, col block t // P
        self.DBLK = (self.TILES + P - 1) // P


_CFG_OVERRIDE = None


def g_cfg():
    if _CFG_OVERRIDE is not None:
        return _CFG_OVERRIDE
    return Cfg(N=50000, E=600000, HD=128, L=4)


# ---------------------------------------------------------------- host prep


def host_prep(cfg, z, pos, edge_index, lam, atom_embed, lam_w, lam_b, edge_w1):
    """Returns per-core data arrays (lists indexed by core) + shared arrays."""
    N, NC, G, CAP, WIN, SWIN = cfg.N, cfg.NC, cfg.G, cfg.CAP, cfg.WIN, cfg.SWIN
    NLOC, NLOCP, CHUNKS, TILES = cfg.NLOC, cfg.NLOCP, cfg.CHUNKS, cfg.TILES

    z = np.asarray(z)
    pos = np.asarray(pos, np.float32)
    ei = np.asarray(edge_index)
    lam = np.float32(np.asarray(lam))
    atom_embed = np.asarray(atom_embed, np.float32)
    lam_vec = lam * np.asarray(lam_w, np.float32)[0] + np.asarray(lam_b, np.float32)
    h0 = atom_embed[z] + lam_vec[None, :]  # [N, HD] f32
    bf16 = ml_dtypes.bfloat16

    row, col = ei[0].astype(np.int64), ei[1].astype(np.int64)
    diff = pos[row] - pos[col]
    dist = np.sqrt((diff * diff).sum(-1) + 1e-8).astype(np.float32)  # [E]

    # padded-global node id (gather-table layout: NC slices of NLOCP rows)
    def gid(n):
        return (n // NLOC) * NLOCP + (n % NLOC)

    order = np.argsort(row, kind="stable")
    row_s, col_s, dist_s = row[order], col[order], dist[order]
    core_of = row_s // NLOC

    # h0 full padded table (bf16), identical for every core
    h0full = np.zeros((NC * NLOCP, cfg.HD), np.float32)
    for c in range(NC):
        h0full[c * NLOCP : c * NLOCP + NLOC] = h0[c * NLOC : (c + 1) * NLOC]
    h0full_bf = h0full.astype(bf16)

    w1a_0 = np.asarray(edge_w1[0], np.float32)[: cfg.HD]  # [HD, HD]
    w1b_0 = np.asarray(edge_w1[0], np.float32)[cfg.HD : 2 * cfg.HD]
    y0full = (h0full_bf.astype(np.float32) @ w1b_0).astype(bf16)

    out = dict(colidx=[], dist=[], relcol=[], h0slice=[], h0sliceT=[], xloc0=[])
    for c in range(NC):
        m = core_of == c
        r = row_s[m] - c * NLOC
        cl = col_s[m]
        dd = dist_s[m]
        ne = len(r)
        ch_rows = np.zeros((CHUNKS, CAP), np.int64)
        ch_cols = np.zeros((CHUNKS, CAP), np.int64)
        ch_dist = np.zeros((CHUNKS, CAP), np.float32)
        ch_valid = np.zeros((CHUNKS, CAP), bool)
        cur = 0
        for k in range(CHUNKS):
            node_end = min((k + 1) * G, NLOC)
            n_avail = int(np.searchsorted(r, node_end, side="left")) - cur
            take = min(CAP, n_avail)
            if take > 0:
                sl = slice(cur, cur + take)
                assert r[sl].min() >= max(0, cfg.o(k)), (
                    f"core {c} chunk {k}: row {r[sl].min()} < window {cfg.o(k)}"
                )
                ch_rows[k, :take] = r[sl]
                ch_cols[k, :take] = cl[sl]
                ch_dist[k, :take] = dd[sl]
                ch_valid[k, :take] = True
                cur += take
        assert cur == ne, f"core {c}: {ne - cur} edges unassigned; schedule infeasible"

        # gather indices (padded-global), [P, CHUNKS] for multi-col offsets
        gidx = np.where(ch_valid, gid(ch_cols), 0).astype(np.int32)
        out["colidx"].append(np.ascontiguousarray(gidx.T))  # [128, CHUNKS]

        # dist columns: [P, CHUNKS] bf16 (edge e of chunk c -> dist)
        out["dist"].append(np.ascontiguousarray(ch_dist.T).astype(bf16))

        # rel-index table for one-hot gen: [P, CHUNKS] f32, rel = row - sbase(t)
        relc = np.full((CHUNKS, CAP), -1000.0, np.float32)
        t_idx = np.arange(CHUNKS) // 4
        for k in range(CHUNKS):
            v = ch_valid[k]
            rel = (ch_rows[k] - cfg.sbase(t_idx[k])).astype(np.float32)
            assert np.all((rel[v] >= 0) & (rel[v] < SWIN)), f"S window overflow c{c} k{k}"
            # scatter window containment: rel2 = row - o(k) in [0, WIN)
            rel2 = ch_rows[k] - cfg.o(k)
            assert np.all((rel2[v] >= 0) & (rel2[v] < WIN)), f"S2 overflow c{c} k{k}"
            relc[k, v] = rel[v]
        out["relcol"].append(np.ascontiguousarray(relc.T))  # [128, CHUNKS]

        hs = np.zeros((NLOCP, cfg.HD), np.float32)
        hs[:NLOC] = h0[c * NLOC : (c + 1) * NLOC]
        out["h0slice"].append(hs.astype(bf16))
        out["h0sliceT"].append(np.ascontiguousarray(hs.T).astype(bf16))

        # layer-0 xloc, padded DRAM image [XROWS, HD]
        x0 = np.zeros((cfg.XROWS, cfg.HD), np.float32)
        x0[cfg.XPAD : cfg.XPAD + NLOCP] = hs.astype(bf16).astype(np.float32) @ w1a_0
        out["xloc0"].append(x0.astype(bf16))

    shared = {"y0full": y0full}
    return out, shared


def host_weights(cfg, lam, lam_w, lam_b, edge_w1, edge_b1, edge_w2, edge_b2,
                 node_w1, node_b1, node_w2, node_b2, ln_g, ln_b,
                 head_w1, head_b1, head_w2, head_b2):
    f = lambda x: np.asarray(x, np.float32)
    bf16 = ml_dtypes.bfloat16
    lam = np.float32(np.asarray(lam))
    lam_vec = lam * f(lam_w)[0] + f(lam_b)
    W = {}
    HD = cfg.HD
    for i in range(cfg.L):
        w1 = f(edge_w1[i])  # [3*HD+1, HD]
        W[f"W1a_{i}"] = w1[:HD].astype(bf16)
        W[f"W1b_{i}"] = w1[HD : 2 * HD].astype(bf16)
        W[f"wd_{i}"] = np.ascontiguousarray(w1[2 * HD : 2 * HD + 1]).astype(bf16)
        b1e = f(edge_b1[i]) + lam_vec @ w1[2 * HD + 1 :]
        W[f"b1_{i}"] = b1e[:, None].astype(np.float32)  # [HD,1]
        W[f"W2_{i}"] = f(edge_w2[i]).astype(bf16)
        W[f"b2r_{i}"] = f(edge_b2[i])[None, :].astype(np.float32)  # [1,HD]
        nw1 = f(node_w1[i])
        W[f"nw1a_{i}"] = nw1[:HD].astype(bf16)
        W[f"nw1b_{i}"] = nw1[HD:].astype(bf16)
        W[f"nb1_{i}"] = f(node_b1[i])[:, None].astype(np.float32)
        W[f"nw2_{i}"] = f(node_w2[i]).astype(bf16)
        W[f"nb2r_{i}"] = f(node_b2[i])[None, :].astype(np.float32)
        W[f"g_{i}"] = np.broadcast_to(f(ln_g[i])[None, :], (P, HD)).astype(bf16).copy()
        W[f"b_{i}"] = np.broadcast_to(f(ln_b[i])[None, :], (P, HD)).astype(bf16).copy()
    W["hw1"] = f(head_w1)  # [HD,HD] f32
    W["hb1r"] = f(head_b1)[None, :]  # [1,HD]
    W["hw2"] = f(head_w2)  # [HD,1]
    W["hb2"] = f(head_b2)[None, :]  # [1,1]
    mask = np.zeros((cfg.NB * P,), np.float32)
    mask[: cfg.NLOC] = 1.0
    W["pmask"] = np.ascontiguousarray(mask.reshape(cfg.NB, P).T).astype(bf16)
    # iota for one-hot gen: [P, SWIN] bf16, every row = 0..SWIN-1
    W["iota"] = np.broadcast_to(
        np.arange(cfg.SWIN, dtype=np.float32)[None, :], (P, cfg.SWIN)
    ).astype(bf16).copy()
    triv = {
        "b2": all(not np.any(f(edge_b2[i])) for i in range(cfg.L)),
        "nb2": all(not np.any(f(node_b2[i])) for i in range(cfg.L)),
        "gb": all(
            np.all(f(ln_g[i]) == 1.0) and not np.any(f(ln_b[i])) for i in range(cfg.L)
        ),
        "hb1": not np.any(f(head_b1)),
        "hb2": not np.any(f(head_b2)),
    }
    return W, triv


# ------------------------------------------------------------- device program


def sw_chunk_ranges(cfg):
    """Static: for each superwindow s, the chunk range [first, last] touching it."""
    out = []
    for s in range(cfg.NSW):
        lo, hi = 512 * s, 512 * (s + 1)
        cs = [
            c
            for c in range(cfg.CHUNKS)
            if max(0, cfg.o(c)) < hi and cfg.o(c) + cfg.WIN > lo
        ]
        out.append((min(cs), max(cs)) if cs else None)
    return out


def build_program(cfg, triv, skip=()):
    skip = set(skip)
    NB, TILES, CHUNKS, WIN, SWIN, GT = (
        cfg.NB, cfg.TILES, cfg.CHUNKS, cfg.WIN, cfg.SWIN, cfg.GT
    )
    L, NLOCP, NC = cfg.L, cfg.NLOCP, cfg.NC
    nc = bacc.Bacc("TRN2", debug=False, num_devices=NC, target_bir_lowering=False)

    # ---- dram I/O
    d_colidx = nc.dram_tensor("colidx", [P, CHUNKS], I32, kind="ExternalInput")
    d_dist = nc.dram_tensor("dist", [P, CHUNKS], BF16, kind="ExternalInput")
    d_relcol = nc.dram_tensor("relcol", [P, CHUNKS], F32, kind="ExternalInput")
    d_h0 = nc.dram_tensor("h0slice", [NLOCP, P], BF16, kind="ExternalInput")
    d_h0T = nc.dram_tensor("h0sliceT", [P, NLOCP], BF16, kind="ExternalInput")
    d_x0 = nc.dram_tensor("xloc0", [cfg.XROWS, P], BF16, kind="ExternalInput")
    d_y0full = nc.dram_tensor("y0full", [NC * NLOCP, P], BF16, kind="ExternalInput")
    wnames = (
        [f"{n}_{i}" for i in range(L) for n in
         ("W1a", "W1b", "W2", "nw1a", "nw1b", "nw2", "g", "b")]
    )
    d_w = {n: nc.dram_tensor(n, [P, P], BF16, kind="ExternalInput") for n in wnames}
    for i in range(L):
        d_w[f"wd_{i}"] = nc.dram_tensor(f"wd_{i}", [1, P], BF16, kind="ExternalInput")
        d_w[f"b1_{i}"] = nc.dram_tensor(f"b1_{i}", [P, 1], F32, kind="ExternalInput")
        d_w[f"nb1_{i}"] = nc.dram_tensor(f"nb1_{i}", [P, 1], F32, kind="ExternalInput")
        d_w[f"b2r_{i}"] = nc.dram_tensor(f"b2r_{i}", [1, P], F32, kind="ExternalInput")
        d_w[f"nb2r_{i}"] = nc.dram_tensor(f"nb2r_{i}", [1, P], F32, kind="ExternalInput")
    d_w["hw1"] = nc.dram_tensor("hw1", [P, P], F32, kind="ExternalInput")
    d_w["hb1r"] = nc.dram_tensor("hb1r", [1, P], F32, kind="ExternalInput")
    d_w["hw2"] = nc.dram_tensor("hw2", [P, 1], F32, kind="ExternalInput")
    d_w["hb2"] = nc.dram_tensor("hb2", [1, 1], F32, kind="ExternalInput")
    d_w["pmask"] = nc.dram_tensor("pmask", [P, NB], BF16, kind="ExternalInput")
    d_w["iota"] = nc.dram_tensor("iota", [P, SWIN], BF16, kind="ExternalInput")
    d_out = nc.dram_tensor("out_y", [1, 1], F32, kind="ExternalOutput")
    dbg_layer = int(os.environ.get("K_DBG_LAYER", "-1"))
    d_dbg = None
    if dbg_layer >= 0:
        d_dbg = nc.dram_tensor("out_dbg", [NLOCP, P], F32, kind="ExternalOutput")
        d_dbga = nc.dram_tensor("out_dbga", [NLOCP, P], F32, kind="ExternalOutput")

    # internal dram (xloc + hfull double-buffered by layer parity)
    d_xloc = [nc.dram_tensor(f"xlocbuf{i}", [cfg.XROWS, P], BF16) for i in range(2)]
    d_yslice = nc.dram_tensor("yslice", [NLOCP, P], BF16)
    d_yfull = [
        nc.dram_tensor(f"yfull{i}", [NC * NLOCP, P], BF16, addr_space="Shared")
        for i in range(2)
    ]
    d_pool = nc.dram_tensor("poolpart", [1, P], F32)
    d_poolr = nc.dram_tensor("poolred", [1, P], F32, addr_space="Shared")

    groups = [list(range(NC))]
    swr = sw_chunk_ranges(cfg)
    # tile at which superwindow s flushes (last chunk's tile)
    flush_tile = {}
    for s in range(cfg.NSW):
        if swr[s]:
            flush_tile.setdefault(swr[s][1] // 4, []).append(s)

    with ExitStack() as ctx:
        tc = ctx.enter_context(tile.TileContext(nc))
        cst = ctx.enter_context(tc.tile_pool(name="cst", bufs=1))
        res = ctx.enter_context(tc.tile_pool(name="res", bufs=1))
        sbe = ctx.enter_context(tc.tile_pool(name="sbe", bufs=4))
        sbg = ctx.enter_context(tc.tile_pool(name="sbg", bufs=6))
        sbn = ctx.enter_context(tc.tile_pool(name="sbn", bufs=3))
        ps = ctx.enter_context(tc.tile_pool(name="ps", bufs=2, space="PSUM"))
        pagg = ctx.enter_context(tc.tile_pool(name="pagg", bufs=2, space="PSUM"))

        # ---- resident constants
        ident = cst.tile([P, P], BF16)
        make_identity(nc, ident[:])
        colidx = cst.tile([P, CHUNKS], I32)
        nc.sync.dma_start(colidx[:], d_colidx[:])
        dist_col = cst.tile([P, CHUNKS], BF16)
        nc.sync.dma_start(dist_col[:], d_dist[:])
        relcol = cst.tile([P, CHUNKS], F32)
        nc.sync.dma_start(relcol[:], d_relcol[:])
        iota = cst.tile([P, SWIN], BF16)
        nc.sync.dma_start(iota[:], d_w["iota"][:])
        onesf = cst.tile([1, 1], F32)
        nc.vector.memset(onesf[:], 1.0)
        onesf_col = cst.tile([1, P], F32)
        nc.vector.memset(onesf_col[:], 1.0)
        pmask = cst.tile([P, NB], BF16)
        nc.sync.dma_start(pmask[:], d_w["pmask"][:])
        eps_t = cst.tile([P, 1], F32)
        nc.vector.memset(eps_t[:], 1e-5)

        wt = {}
        for n in wnames:
            wt[n] = cst.tile([P, P], BF16, tag=f"w_{n}", name=f"w_{n}")
            nc.sync.dma_start(wt[n][:], d_w[n][:])
        for i in range(L):
            for n, shp, dt in (
                (f"wd_{i}", [1, P], BF16),
                (f"b1_{i}", [P, 1], F32),
                (f"nb1_{i}", [P, 1], F32),
                (f"b2r_{i}", [1, P], F32),
                (f"nb2r_{i}", [1, P], F32),
            ):
                wt[n] = cst.tile(shp, dt, tag=f"w_{n}", name=f"w_{n}")
                nc.sync.dma_start(wt[n][:], d_w[n][:])
        for n, shp in (("hw1", [P, P]), ("hb1r", [1, P]), ("hw2", [P, 1]), ("hb2", [1, 1])):
            wt[n] = cst.tile(shp, F32, tag=f"w_{n}", name=f"w_{n}")
            nc.sync.dma_start(wt[n][:], d_w[n][:])

        # resident state
        hT = res.tile([P, NLOCP], BF16)
        hnat = res.tile([P, NB * P], BF16)  # [p, (b, d)]
        aggT = res.tile([P, NLOCP], BF16)
        nc.vector.memset(aggT[:], 0.0)
        nc.sync.dma_start(hT[:], d_h0T[:])
        for b in range(NB):
            nc.sync.dma_start(hnat[:, ts(b, P)], d_h0[b * P : (b + 1) * P, :])

        def node_block(li, b, agg_pool):
            """Node MLP + LN for block b of layer li; also next-layer xloc."""
            last = li == L - 1
            nw1a, nw1b = wt[f"nw1a_{li}"], wt[f"nw1b_{li}"]
            nb1, nw2 = wt[f"nb1_{li}"], wt[f"nw2_{li}"]
            u1 = ps.tile([P, 512], F32, tag="m1")
            nc.tensor.matmul(u1[:, :P], nw1a[:], hT[:, ts(b, P)], start=True, stop=False)
            nc.tensor.matmul(u1[:, :P], nw1b[:], aggT[:, ts(b, P)], start=False, stop=True)
            u1sb = sbn.tile([P, P], BF16, tag="u1sb")
            nc.scalar.activation(u1sb[:], u1[:, :P], AF.Silu, bias=nb1[:])

            u2 = ps.tile([P, 512], F32, tag="m2")
            nc.tensor.matmul(u2[:, :P], u1sb[:], nw2[:], start=True, stop=triv["nb2"])
            if not triv["nb2"]:
                nc.tensor.matmul(
                    u2[:, :P], onesf_col[:1, :], wt[f"nb2r_{li}"][:1, :],
                    start=False, stop=True,
                )
            x = sbn.tile([P, P], F32, tag="x")
            nc.vector.tensor_tensor(
                out=x[:], in0=hnat[:, ts(b, P)], in1=u2[:, :P], op=ALU.add
            )
            red = sbn.tile([P, 1], F32, tag="red")
            nc.vector.tensor_reduce(red[:], x[:], axis=mybir.AxisListType.X, op=ALU.add)
            negmu = sbn.tile([P, 1], F32, tag="negmu")
            nc.vector.tensor_scalar_mul(negmu[:], red[:], -1.0 / P)
            xsq = ps.tile([P, 512], F32, tag="m1")
            ssq = sbn.tile([P, 1], F32, tag="ssq")
            nc.scalar.activation(
                xsq[:, :P], x[:], AF.Square, bias=negmu[:], accum_out=ssq[:]
            )
            sd = sbn.tile([P, 1], F32, tag="sd")
            nc.scalar.activation(sd[:], ssq[:], AF.Sqrt, scale=1.0 / P, bias=eps_t[:])
            rstd = sbn.tile([P, 1], F32, tag="rstd")
            nc.vector.reciprocal(rstd[:], sd[:])
            ydst = hnat[:, ts(b, P)]
            if triv["gb"]:
                nc.vector.tensor_scalar(
                    ydst, x[:], scalar1=negmu[:], scalar2=rstd[:],
                    op0=ALU.add, op1=ALU.mult,
                )
            else:
                y = sbn.tile([P, P], BF16, tag="y")
                nc.vector.tensor_scalar(
                    y[:], x[:], scalar1=negmu[:], scalar2=rstd[:],
                    op0=ALU.add, op1=ALU.mult,
                )
                yg = sbn.tile([P, P], BF16, tag="yg")
                nc.vector.tensor_tensor(
                    out=yg[:], in0=y[:], in1=wt[f"g_{li}"][:], op=ALU.mult
                )
                nc.vector.tensor_tensor(
                    out=ydst, in0=yg[:], in1=wt[f"b_{li}"][:], op=ALU.add
                )
            # update hT (transpose via bf16 psum tag tr)
            trp = ps.tile([P, 1024], BF16, tag="tr")
            nc.tensor.transpose(trp[:, :P], ydst, ident[:])
            nc.vector.tensor_copy(hT[:, ts(b, P)], trp[:, :P])
            if not last:
                # next-layer xloc (= h @ W1a') and yloc (= h @ W1b')
                xps = ps.tile([P, 512], F32, tag="m2")
                nc.tensor.matmul(
                    xps[:, :P], hT[:, ts(b, P)], wt[f"W1a_{li + 1}"][:],
                    start=True, stop=True,
                )
                nc.tensor.matmul(
                    xps[:, P : 2 * P], hT[:, ts(b, P)], wt[f"W1b_{li + 1}"][:],
                    start=True, stop=True,
                )
                xsb = sbn.tile([P, 2 * P], BF16, tag="xsb")
                nc.vector.tensor_copy(xsb[:], xps[:, : 2 * P])
                nc.sync.dma_start(
                    d_xloc[(li + 1) % 2][cfg.XPAD + b * P : cfg.XPAD + (b + 1) * P, :],
                    xsb[:, :P],
                )
                nc.sync.dma_start(
                    d_yslice[b * P : (b + 1) * P, :], xsb[:, P : 2 * P]
                )

        # zero xloc dram pad regions once (both buffers)
        zrow128 = cst.tile([P, P], BF16)
        nc.vector.memset(zrow128[:], 0.0)
        zrow512 = cst.tile([1, 512], BF16)
        nc.vector.memset(zrow512[:], 0.0)
        for xb in range(2):
            nc.sync.dma_start(d_xloc[xb][: cfg.XPAD, :], zrow128[: cfg.XPAD, :])
            r0 = cfg.XPAD + NLOCP
            while r0 < cfg.XROWS:
                n = min(P, cfg.XROWS - r0)
                nc.sync.dma_start(d_xloc[xb][r0 : r0 + n, :], zrow128[:n, :])
                r0 += n

        for li in range(L):
            W1b, W2 = wt[f"W1b_{li}"], wt[f"W2_{li}"]
            wd, b1 = wt[f"wd_{li}"], wt[f"b1_{li}"]
            xsrc = d_x0 if li == 0 else d_xloc[li % 2]
            ysrc = d_y0full if li == 0 else d_yfull[li % 2]
            last = li == L - 1
            if last:
                agg_pool = None

            agg_tiles = {}
            gat = None
            done_blocks = 0
            for t in range(TILES if "edge" not in skip else 0):
                gi = 0
                gat = sbg.tile([P, 512], BF16, tag="gat")
                if "gather" not in skip:
                    for j in range(4):
                        c = 4 * t + j
                        nc.gpsimd.indirect_dma_start(
                            out=gat[:, ts(j, P)],
                            out_offset=None,
                            in_=ysrc[:],
                            in_offset=bass.IndirectOffsetOnAxis(
                                ap=colidx[:, c : c + 1], axis=0
                            ),
                        )
                else:
                    nc.vector.memset(gat[:], 0.0)

                # xloc window + wd row (expand stationary, SWIN+1 rows)
                xw = sbe.tile([SWIN + 1, P], BF16, tag="xw")
                x0p = cfg.sbase(t) + cfg.XPAD
                nc.sync.dma_start(xw[:SWIN, :], xsrc[x0p : x0p + SWIN, :])
                nc.vector.tensor_copy(xw[SWIN : SWIN + 1, :], wd[:1, :])

                # one-hot gen [128e, SWIN+1] per chunk; col SWIN = dist
                SW1 = SWIN + 1
                one = sbe.tile([P, 4 * SW1], BF16, tag="one")
                for j in range(4):
                    c = 4 * t + j
                    nc.vector.tensor_scalar(
                        one[:, j * SW1 : j * SW1 + SWIN],
                        iota[:],
                        scalar1=relcol[:, c : c + 1],
                        scalar2=None,
                        op0=ALU.is_equal,
                    )
                nc.vector.tensor_copy(
                    one[:, SWIN : 4 * SW1 : SW1], dist_col[:, 4 * t : 4 * t + 4]
                )
                # one-hot transposes into bf16 psum, then to SBUF
                trp = ps.tile([P, 1024], BF16, tag="tr")
                for j in range(4):
                    nc.tensor.transpose(
                        trp[:SW1, ts(j, P)],
                        one[:, j * SW1 : (j + 1) * SW1],
                        ident[:],
                    )
                S_sb = sbe.tile([SW1, 512], BF16, tag="S")
                nc.vector.tensor_copy(S_sb[:], trp[:SW1, :512])

                m1 = ps.tile([P, 512], F32, tag="m1")
                nc.tensor.matmul(m1[:], xw[:], S_sb[:], start=True, stop=False)
                for j in range(4):
                    nc.tensor.matmul(
                        m1[:, ts(j, P)],
                        gat[:, ts(j, P)],
                        ident[:],
                        start=False,
                        stop=(j == 3),
                        skip_group_check=True,
                    )
                m1sb = sbe.tile([P, 512], BF16, tag="m1sb")
                nc.scalar.activation(m1sb[:], m1[:], AF.Silu, bias=b1[:])

                m2 = ps.tile([P, 512], F32, tag="m2")
                for j in range(4):
                    nc.tensor.matmul(
                        m2[:, ts(j, P)], m1sb[:, ts(j, P)], W2[:],
                        start=True, stop=triv["b2"],
                    )
                    if not triv["b2"]:
                        nc.tensor.matmul(
                            m2[:, ts(j, P)], onesf_col[:1, :], wt[f"b2r_{li}"][:1, :],
                            start=False, stop=True,
                        )
                msb = sbe.tile([P, 512], BF16, tag="msb")
                nc.scalar.activation(msb[:], m2[:], AF.Silu)

                # scatter into superwindows; one-hot slice of `one`
                for j in range(4 if "scatter" not in skip else 0):
                    c = 4 * t + j
                    a0, b0 = max(0, cfg.o(c)), cfg.o(c) + WIN
                    off = cfg.o(c) - cfg.sbase(t)  # = 10*j + 2
                    for s in range(cfg.NSW):
                        lo, hi = max(a0, 512 * s), min(b0, 512 * (s + 1))
                        if lo >= hi:
                            continue
                        if s not in agg_tiles:
                            agg_tiles[s] = pagg.tile(
                                [P, 512], F32, tag="agg", name=f"agg_sw{li}_{s}"
                            )
                            nc.tensor.matmul(
                                agg_tiles[s][:], zrow128[:1, :], zrow512[:1, :],
                                start=True, stop=False,
                            )
                        nc.tensor.matmul(
                            agg_tiles[s][:, lo - 512 * s : hi - 512 * s],
                            msb[:, ts(j, P)],
                            one[:, j * (SWIN + 1) + off + lo - cfg.o(c) :
                                j * (SWIN + 1) + off + hi - cfg.o(c)],
                            start=False,
                            stop=(c == min(swr[s][1], CHUNKS - 1)),
                        )
                # flush superwindows ending at this tile; release node blocks
                for s in flush_tile.get(t, []) if "scatter" not in skip else []:
                    wdt = min(512, NLOCP - 512 * s)
                    if wdt > 0 and s in agg_tiles:
                        nc.vector.tensor_copy(
                            aggT[:, 512 * s : 512 * s + wdt], agg_tiles[s][:, :wdt]
                        )
                    if s in agg_tiles:
                        del agg_tiles[s]
                    nb_ready = min(NB, ((512 * s + wdt) // P))
                    if "node" not in skip:
                        while done_blocks < nb_ready:
                            node_block(li, done_blocks, None)
                            done_blocks += 1

            if "node" not in skip:
                while done_blocks < NB:
                    node_block(li, done_blocks, None)
                    done_blocks += 1

            if li == dbg_layer:
                for b in range(NB):
                    dsb = sbn.tile([P, P], F32, tag="dsb", name=f"dsb{b}")
                    nc.vector.tensor_copy(dsb[:], hnat[:, ts(b, P)])
                    nc.sync.dma_start(d_dbg[b * P : (b + 1) * P, :], dsb[:])
                    dsa = sbn.tile([P, P], F32, tag="dsa", name=f"dsa{b}")
                    nc.vector.tensor_copy(dsa[:], aggT[:, ts(b, P)])
                    nc.sync.dma_start(d_dbga[b * P : (b + 1) * P, :], dsa[:])

            if not last and "cc" not in skip:
                nc.gpsimd.collective_compute(
                    "AllGather", ALU.bypass, replica_groups=groups,
                    ins=[d_yslice[:]], outs=[d_yfull[(li + 1) % 2][:]],
                )

        # ---- pooled mean + head
        agg_pool = pagg.tile([P, 512], F32, tag="agg")
        for b in range(NB):
            nc.tensor.matmul(
                agg_pool[:1, :P], pmask[:, b : b + 1], hnat[:, ts(b, P)],
                start=(b == 0), stop=(b == NB - 1),
            )
        pool_sb = sbn.tile([1, P], F32, tag="pool_sb")
        nc.vector.tensor_scalar_mul(pool_sb[:], agg_pool[:1, :P], 1.0 / cfg.N)
        nc.sync.dma_start(d_pool[:], pool_sb[:])
        nc.gpsimd.collective_compute(
            "AllReduce", ALU.add, replica_groups=groups,
            ins=[d_pool[:]], outs=[d_poolr[:]],
        )
        pT = sbn.tile([P, 1], F32, tag="pT")
        nc.sync.dma_start(pT[:], d_poolr.rearrange("o d -> d o"))
        p1 = ps.tile([P, 512], F32, tag="m1")
        nc.tensor.matmul(p1[:1, :P], pT[:], wt["hw1"][:], start=True, stop=triv["hb1"])
        if not triv["hb1"]:
            nc.tensor.matmul(p1[:1, :P], onesf[:], wt["hb1r"][:], start=False, stop=True)
        p1sb = sbn.tile([1, P], F32, tag="p1sb")
        nc.scalar.activation(p1sb[:], p1[:1, :P], AF.Silu)
        p1T = ps.tile([P, 512], F32, tag="m2")
        nc.tensor.matmul(p1T[:, :1], p1sb[:], onesf[:], start=True, stop=True)
        p1T_sb = sbn.tile([P, 1], F32, tag="p1T_sb")
        nc.vector.tensor_copy(p1T_sb[:], p1T[:, :1])
        yps = ps.tile([P, 512], F32, tag="m1")
        nc.tensor.matmul(yps[:1, :1], p1T_sb[:], wt["hw2"][:], start=True, stop=triv["hb2"])
        if not triv["hb2"]:
            nc.tensor.matmul(yps[:1, :1], onesf[:], wt["hb2"][:], start=False, stop=True)
        ysb = sbn.tile([1, 1], F32, tag="ysb")
        nc.vector.tensor_copy(ysb[:], yps[:1, :1])
        nc.sync.dma_start(d_out[:], ysb[:])

    nc.compile()
    return nc


# ------------------------------------------------------------------ kernel()

_cache = {}


def _get_program(cfg, triv_key, triv):
    skip = tuple(x for x in os.environ.get("K_SKIP", "").split(",") if x)
    key = (cfg.N, cfg.E, cfg.L, triv_key, skip)
    if key not in _cache:
        _cache[key] = build_program(cfg, triv, skip=skip)
    return _cache[key]


def prepare(inputs):
    cfg = g_cfg()
    data, shared = host_prep(
        cfg,
        inputs["z"], inputs["pos"], inputs["edge_index"], inputs["lam"],
        inputs["atom_embed"], inputs["lam_w"], inputs["lam_b"], inputs["edge_w1"],
    )
    W, triv = host_weights(
        cfg, inputs["lam"], inputs["lam_w"], inputs["lam_b"],
        inputs["edge_w1"], inputs["edge_b1"], inputs["edge_w2"], inputs["edge_b2"],
        inputs["node_w1"], inputs["node_b1"], inputs["node_w2"], inputs["node_b2"],
        inputs["ln_g"], inputs["ln_b"],
        inputs["head_w1"], inputs["head_b1"], inputs["head_w2"], inputs["head_b2"],
    )
    triv_key = tuple(sorted(triv.items()))
    nc = _get_program(cfg, triv_key, triv)

    in_maps = []
    for c in range(cfg.NC):
        m = {
            "colidx": data["colidx"][c],
            "dist": data["dist"][c],
            "relcol": data["relcol"][c],
            "h0slice": data["h0slice"][c],
            "h0sliceT": data["h0sliceT"][c],
            "xloc0": data["xloc0"][c],
            "y0full": shared["y0full"],
        }
        for k, v in W.items():
            m[k] = v
        in_maps.append(m)

    return nc, in_maps, cfg


def kernel(**inputs) -> np.ndarray:
    nc, in_maps, cfg = prepare(inputs)
    res = run_bass_kernel_spmd(nc, in_maps, core_ids=list(range(cfg.NC)))
    return res.results[0]["out_y"].reshape(1, 1).astype(np.float32)


if __name__ == "__main__":
    import sys
    sys.path.insert(0, "/root/problem")
    import jax
    cpu = jax.devices("cpu")[0]
    with jax.default_device(cpu):
        from reference import setup_inputs, reference
        inp = setup_inputs()
        inp = {k: np.asarray(v) for k, v in inp.items()}
        exp = np.asarray(reference(**{k: jax.device_put(v, cpu) for k, v in inp.items()}))
    got = kernel(**inp)
    rel = abs(got.ravel()[0] - exp.ravel()[0]) / (abs(exp.ravel()[0]) + 1e-12)
    print(f"expected {exp.ravel()[0]:.6e}  got {got.ravel()[0]:.6e}  rel {rel:.3e}")
